# revision 3
# baseline (speedup 1.0000x reference)
# HDC classifier kernel for Trainium2 (Bass/Tile), data-parallel over 8 NeuronCores.
#
# Per core (B_local = 1024 rows of the batch):
#   projT chunk [dn<=128 of D, 512 batch] = RP_chunk.T @ featT      (PE, 2x bf16 hi/lo passes)
#   hvT enc chunk = binarize(projT)                                 (ACT: sign -> {-1,0,1};
#                                                                    DVE: (x>0)*2 -> {0,2})
#   dot_acc [100, 512] += QT_chunk.T @ hvT_chunk                    (PE, bf16, Q = 2*proto-1)
# dot_acc = 2*dotQ - sumQ_act(c); sims/pred keys are per-class affine maps of dot_acc,
# integer-exact in fp32 so argmax tie-breaking matches jnp.argmax (first index wins,
# via reversed-iota max trick).
#
# kernel(**inputs) takes FULL inputs as in reference.setup_inputs() and returns
# (preds int32 [8192], sims float32 [8192, 100]).

import numpy as np
from contextlib import ExitStack

import concourse.bass as bass
import concourse.tile as tile
from concourse import bacc, mybir
from concourse.bass_utils import run_bass_kernel_spmd
from concourse.masks import make_identity

dt = mybir.dt
op = mybir.AluOpType
AF = mybir.ActivationFunctionType
AX = mybir.AxisListType

B, F, D, C = 8192, 128, 10000, 100
NCORES = 8
BL = B // NCORES          # 1024 batch rows per core
BN = 512                  # batch block (matmul moving free dim)
NBLK = BL // BN           # 2
P = 128
CHUNKS = [(k * P, min(P, D - k * P)) for k in range((D + P - 1) // P)]  # 79 chunks
NCH = len(CHUNKS)
N_ACT = 53                # chunks [0, N_ACT) binarized on ScalarE, rest on VectorE
ACT_COLS = sum(dn for _, dn in CHUNKS[:N_ACT])
BIG = 1000.0
INV_D = 1.0 / D


def _emit(nc, tc, ctx, aps):
    feats, rp, proto, cc, preds_o, sims_o = aps

    pool_const = ctx.enter_context(tc.tile_pool(name="constp", bufs=1))
    pool_w = ctx.enter_context(tc.tile_pool(name="wts", bufs=1))
    pool_stage = ctx.enter_context(tc.tile_pool(name="stage", bufs=4))
    pool_hv = ctx.enter_context(tc.tile_pool(name="hv", bufs=4))
    pool_ep = ctx.enter_context(tc.tile_pool(name="ep", bufs=2))
    pool_ps1 = ctx.enter_context(tc.tile_pool(name="ps1", bufs=3, space="PSUM"))
    pool_ps2 = ctx.enter_context(tc.tile_pool(name="ps2", bufs=2, space="PSUM"))
    pool_pst = ctx.enter_context(tc.tile_pool(name="pst", bufs=2, space="PSUM"))

    def body():
        # ---------------- constants ----------------
        id_f = pool_const.tile([P, P], dt.float32, tag="idf", name="idf")
        make_identity(nc, id_f[:, :])
        id_b = pool_const.tile([P, P], dt.bfloat16, tag="idb", name="idb")
        make_identity(nc, id_b[:, :])
        negio_i = pool_const.tile([P, C], dt.int32, tag="negio_i", name="negio_i")
        nc.gpsimd.iota(negio_i[:, :], pattern=[[-1, C]], base=int(BIG),
                       channel_multiplier=0)
        negio = pool_const.tile([P, C], dt.float32, tag="negio", name="negio")
        nc.vector.tensor_copy(negio[:, :], negio_i[:, :])

        ccs = pool_const.tile([C, 1], dt.float32, tag="ccs", name="ccs")
        nc.sync.dma_start(ccs[:, :], cc[:, :])
        mask = pool_const.tile([C, 1], dt.float32, tag="mask", name="mask")
        nc.vector.tensor_scalar(out=mask[:, :], in0=ccs[:, :], scalar1=0.0,
                                scalar2=None, op0=op.is_gt)

        # ---------------- prototypes -> Q (bf16) + per-class sums ----------------
        proto_u8 = pool_const.tile([C, D], dt.uint8, tag="proto_u8", name="proto_u8")
        nc.sync.dma_start(proto_u8[:, :], proto[:, :])
        q_bf = pool_const.tile([C, D], dt.bfloat16, tag="q_bf", name="q_bf")
        sumA = pool_const.tile([C, 1], dt.float32, tag="sumA", name="sumA")
        sumB = pool_const.tile([C, 1], dt.float32, tag="sumB", name="sumB")
        # Q = 2*p - 1 on two engines; accum_out gives per-class column sums of Q
        # NOTE: DVE tensor_scalar mis-handles uint8 inputs (op1 add not applied),
        # so both halves run on ScalarE.
        nc.scalar.activation(q_bf[:, 0:ACT_COLS], proto_u8[:, 0:ACT_COLS],
                             AF.Copy, bias=-1.0, scale=2.0, accum_out=sumA[:, :])
        nc.scalar.activation(q_bf[:, ACT_COLS:D], proto_u8[:, ACT_COLS:D],
                             AF.Copy, bias=-1.0, scale=2.0, accum_out=sumB[:, :])

        # epilogue per-class affine constants
        sumT = pool_const.tile([C, 1], dt.float32, tag="sumT", name="sumT")
        nc.vector.tensor_add(sumT[:, :], sumA[:, :], sumB[:, :])
        proto_sum = pool_const.tile([C, 1], dt.float32, tag="psum_c", name="psum_c")
        nc.vector.tensor_scalar(out=proto_sum[:, :], in0=sumT[:, :], scalar1=float(D),
                                scalar2=0.5, op0=op.add, op1=op.mult)
        t2 = pool_const.tile([C, 1], dt.float32, tag="t2", name="t2")
        nc.vector.scalar_tensor_tensor(out=t2[:, :], in0=sumA[:, :], scalar=0.5,
                                       in1=proto_sum[:, :], op0=op.mult,
                                       op1=op.subtract)
        nc.vector.tensor_scalar_add(t2[:, :], t2[:, :], float(D))
        s1p = pool_const.tile([C, 1], dt.float32, tag="s1p", name="s1p")
        nc.vector.tensor_scalar_mul(s1p[:, :], mask[:, :], 0.5)
        s2p = pool_const.tile([C, 1], dt.float32, tag="s2p", name="s2p")
        nc.vector.tensor_scalar(out=s2p[:, :], in0=t2[:, :], scalar1=mask[:, :],
                                scalar2=-float(D), op0=op.mult, op1=op.add)
        u = pool_const.tile([C, 1], dt.float32, tag="u", name="u")
        nc.vector.tensor_scalar(out=u[:, :], in0=t2[:, :], scalar1=-float(D),
                                scalar2=INV_D, op0=op.add, op1=op.mult)
        s1s = pool_const.tile([C, 1], dt.float32, tag="s1s", name="s1s")
        nc.vector.tensor_scalar_mul(s1s[:, :], mask[:, :], 0.5 * INV_D)
        s2s = pool_const.tile([C, 1], dt.float32, tag="s2s", name="s2s")
        nc.vector.tensor_scalar(out=s2s[:, :], in0=u[:, :], scalar1=1.0,
                                scalar2=mask[:, :], op0=op.add, op1=op.mult)

        # ---------------- features -> featT hi/lo (bf16) ----------------
        fhi = pool_const.tile([P, BL], dt.bfloat16, tag="fhi", name="fhi")
        flo = pool_const.tile([P, BL], dt.bfloat16, tag="flo", name="flo")
        for j in range(BL // P):
            fstage = pool_stage.tile([P, P], dt.float32, tag="fstage", name="fstage")
            nc.sync.dma_start(fstage[:, :], feats[j * P:(j + 1) * P, :])
            psf = pool_pst.tile([P, P], dt.float32, tag="tps", name="psf")
            nc.tensor.transpose(psf[:, :], fstage[:, :], id_f[:, :])
            sl = slice(j * P, (j + 1) * P)
            nc.vector.tensor_copy(fhi[:, sl], psf[:, :])
            nc.vector.scalar_tensor_tensor(out=flo[:, sl], in0=psf[:, :], scalar=0.0,
                                           in1=fhi[:, sl], op0=op.bypass,
                                           op1=op.subtract)

        # ---------------- RP -> bf16 chunks ----------------
        rp_t = []
        for k, (d0, dn) in enumerate(CHUNKS):
            rpst = pool_stage.tile([P, P], dt.float32, tag="rpst", name="rpst")
            nc.sync.dma_start(rpst[:, 0:dn], rp[:, d0:d0 + dn])
            rpk = pool_w.tile([P, P], dt.bfloat16, tag=f"rp{k}", name=f"rp{k}")
            if k % 2 == 0:
                nc.scalar.copy(rpk[:, 0:dn], rpst[:, 0:dn])
            else:
                nc.vector.tensor_copy(rpk[:, 0:dn], rpst[:, 0:dn])
            rp_t.append(rpk)

        # ---------------- QT chunks (transpose Q) ----------------
        qt_t = []
        for k, (d0, dn) in enumerate(CHUNKS):
            psq = pool_pst.tile([P, C], dt.bfloat16, tag="tps", name="psq")
            nc.tensor.transpose(psq[0:dn, :], q_bf[:, d0:d0 + dn], id_b[0:C, 0:C])
            qtk = pool_w.tile([P, C], dt.bfloat16, tag=f"qt{k}", name=f"qt{k}")
            nc.vector.tensor_copy(qtk[0:dn, :], psq[0:dn, :])
            qt_t.append(qtk)

        # ---------------- main fused loop ----------------
        for blk in range(NBLK):
            bsl = slice(blk * BN, (blk + 1) * BN)
            ps2 = pool_ps2.tile([P, BN], dt.float32, tag="ps2", name="ps2")
            for k, (d0, dn) in enumerate(CHUNKS):
                ps1 = pool_ps1.tile([P, BN], dt.float32, tag="ps1", name="ps1")
                nc.tensor.matmul(ps1[0:dn, :], rp_t[k][:, 0:dn], fhi[:, bsl],
                                 start=True, stop=False)
                nc.tensor.matmul(ps1[0:dn, :], rp_t[k][:, 0:dn], flo[:, bsl],
                                 start=False, stop=True)
                hv = pool_hv.tile([P, BN], dt.bfloat16, tag="hv", name="hv")
                if k < N_ACT:
                    nc.scalar.activation(hv[0:dn, :], ps1[0:dn, :], AF.Sign)
                else:
                    nc.vector.tensor_scalar(out=hv[0:dn, :], in0=ps1[0:dn, :],
                                            scalar1=0.0, scalar2=2.0,
                                            op0=op.is_gt, op1=op.mult)
                nc.tensor.matmul(ps2[0:C, :], qt_t[k][0:dn, 0:C], hv[0:dn, :],
                                 start=(k == 0), stop=(k == NCH - 1))

            # epilogue: per-class affine -> pred keys + sims (both [100, 512])
            pkT = pool_ep.tile([P, BN], dt.float32, tag="pkT", name="pkT")
            nc.vector.tensor_scalar(out=pkT[0:C, :], in0=ps2[0:C, :],
                                    scalar1=s1p[:, :], scalar2=s2p[:, :],
                                    op0=op.mult, op1=op.add)
            smT = pool_ep.tile([P, BN], dt.float32, tag="smT", name="smT")
            nc.vector.tensor_scalar(out=smT[0:C, :], in0=ps2[0:C, :],
                                    scalar1=s1s[:, :], scalar2=s2s[:, :],
                                    op0=op.mult, op1=op.add)
            for j in range(BN // P):
                b0 = blk * BN + j * P
                jsl = slice(j * P, (j + 1) * P)
                pst = pool_pst.tile([P, C], dt.float32, tag="tps", name="pkt_ps")
                nc.tensor.transpose(pst[:, :], pkT[0:C, jsl], id_f[0:C, 0:C])
                pkB = pool_ep.tile([P, C], dt.float32, tag="pkB", name="pkB")
                nc.vector.tensor_copy(pkB[:, :], pst[:, :])
                pst2 = pool_pst.tile([P, C], dt.float32, tag="tps", name="smt_ps")
                nc.tensor.transpose(pst2[:, :], smT[0:C, jsl], id_f[0:C, 0:C])
                smB = pool_ep.tile([P, C], dt.float32, tag="smB", name="smB")
                nc.vector.tensor_copy(smB[:, :], pst2[:, :])
                nc.sync.dma_start(sims_o[b0:b0 + P, :], smB[:, :])

                rmax = pool_ep.tile([P, 1], dt.float32, tag="rmax", name="rmax")
                nc.vector.tensor_reduce(rmax[:, :], pkB[:, :], axis=AX.X, op=op.max)
                msk = pool_ep.tile([P, C], dt.float32, tag="msk", name="msk")
                nc.vector.scalar_tensor_tensor(out=msk[:, :], in0=pkB[:, :],
                                               scalar=rmax[:, :], in1=negio[:, :],
                                               op0=op.is_ge, op1=op.mult)
                m2 = pool_ep.tile([P, 1], dt.float32, tag="m2", name="m2")
                nc.vector.tensor_reduce(m2[:, :], msk[:, :], axis=AX.X, op=op.max)
                pi = pool_ep.tile([P, 1], dt.int32, tag="pi", name="pi")
                nc.vector.tensor_scalar(out=pi[:, :], in0=m2[:, :], scalar1=BIG,
                                        scalar2=-1.0, op0=op.subtract, op1=op.mult)
                nc.sync.dma_start(preds_o[b0:b0 + P, :], pi[:, :])

    return body


def build(repeat: int = 1):
    nc = bacc.Bacc("TRN2", target_bir_lowering=False, debug=False,
                   num_devices=NCORES)
    feats = nc.dram_tensor("features", [BL, F], dt.float32,
                           kind="ExternalInput").ap()
    rp = nc.dram_tensor("random_projection", [F, D], dt.float32,
                        kind="ExternalInput").ap()
    proto = nc.dram_tensor("prototypes", [C, D], dt.uint8,
                           kind="ExternalInput").ap()
    cc = nc.dram_tensor("class_counts", [C, 1], dt.float32,
                        kind="ExternalInput").ap()
    preds_o = nc.dram_tensor("preds", [BL, 1], dt.int32,
                             kind="ExternalOutput").ap()
    sims_o = nc.dram_tensor("sims", [BL, C], dt.float32,
                            kind="ExternalOutput").ap()
    aps = (feats, rp, proto, cc, preds_o, sims_o)

    with tile.TileContext(nc) as tc, ExitStack() as ctx:
        body = _emit(nc, tc, ctx, aps)
        if repeat == 1:
            body()
        else:
            with tc.For_i(0, repeat, 1):
                body()
    nc.compile()
    return nc


def make_in_maps(inputs):
    feats = np.ascontiguousarray(np.asarray(inputs["features"], dtype=np.float32))
    rp = np.ascontiguousarray(np.asarray(inputs["random_projection"],
                                         dtype=np.float32))
    proto = np.ascontiguousarray(
        np.asarray(inputs["prototypes"]).astype(np.uint8))
    cc = np.ascontiguousarray(
        np.asarray(inputs["class_counts"], dtype=np.float32).reshape(C, 1))
    in_maps = []
    for i in range(NCORES):
        in_maps.append({
            "features": feats[i * BL:(i + 1) * BL],
            "random_projection": rp,
            "prototypes": proto,
            "class_counts": cc,
        })
    return in_maps


_built = {}


def get_built(repeat: int = 1):
    if repeat not in _built:
        _built[repeat] = build(repeat)
    return _built[repeat]


def kernel(**inputs):
    nc = get_built(1)
    in_maps = make_in_maps(inputs)
    res = run_bass_kernel_spmd(nc, in_maps, core_ids=list(range(NCORES)))
    preds = np.concatenate(
        [res.results[i]["preds"].reshape(-1) for i in range(NCORES)]
    ).astype(np.int32)
    sims = np.concatenate([res.results[i]["sims"] for i in range(NCORES)], axis=0)
    return preds, sims
